# revision 16
# baseline (speedup 1.0000x reference)
"""Trainium2 Bass kernel for nn_BoundarySeg (segment_reduce).

out[b, j, 0:H]   = sum_{i>=j} A[b, j, i] * h[b, i, :]
out[b, j, H:2H]  = h[b, j, :] * sum_{i>=j} A[b, j, i]

Shapes: A [8, 2048, 2048] f32, h [8, 2048, 256] f32 -> out [8, 2048, 512] f32.
Sharding: data-parallel over batch; core c computes batch c.

Per-core algorithm (L=2048 in 16 tiles of 128, H=256):
  - h loads first as 4 quarters on 4 DMA rings into [128(p), 16(t), 258]
    fp32 with a ones column at [.., 256] (rowsum falls out of the matmul
    as an extra column); V/S cast quarters to bf16 for the matmul rhs.
  - Panels processed in interleaved order [0,15,1,14,...] so per-panel
    work (and the store stream) stays even over time. Panel DMAs go in
    <=8-block f32r chunks alternating sync/scalar HWDGE rings.
  - PE transposes each 128x128 block (f32r in/out, f32r identity as the
    moving operand); PSUM->SBUF move casts to bf16 (V/S alternating,
    4-block groups = 1 PSUM bank); the diagonal block gets the i>=j
    mask in that move. Matmuls (acc += At^T @ h_bf, bf16, N=258) lag
    the transposes by MM_LAG groups so the in-order PE queue never
    waits on the cross-engine copies.
  - Store per panel (deferred one panel to avoid head-of-line blocks):
    first half copy PSUM->SBUF (ACT), second half = h[j,:] * acc[:,256]
    via DVE tensor_scalar reading the rowsum straight from PSUM; out
    DMA on the gpsimd SWDGE ring.
"""

import os
import sys

import numpy as np

sys.path.insert(0, "/opt/trn_rl_repo")

import concourse.bass as bass  # noqa: E402
import concourse.bacc as bacc  # noqa: E402
import concourse.tile as tile  # noqa: E402
from concourse import mybir  # noqa: E402
from concourse.bass_utils import run_bass_kernel_spmd  # noqa: E402
from concourse.masks import make_identity, make_lower_triangular  # noqa: E402

B, L, H = 8, 2048, 256
P = 128
NT = L // P
HE = H + 2  # even N; col H = ones (rowsum), col H+1 unused
DMA_CHUNK = 8  # blocks per A-panel DMA
TGROUP = 4  # blocks per PE-transpose PSUM tile / copy (1 PSUM bank)
MM_LAG = 3  # groups the matmuls trail the transposes by

DT = mybir.dt.float32
F32R = mybir.dt.float32r
BF16 = mybir.dt.bfloat16

LAST_RESULTS = None
_NC_CACHE = {}


def _build_nc():
    nc = bacc.Bacc(None, target_bir_lowering=False)
    a_dram = nc.dram_tensor("a", [L, L], DT, kind="ExternalInput")
    h_dram = nc.dram_tensor("h", [L, H], DT, kind="ExternalInput")
    out_dram = nc.dram_tensor("out", [L, 2 * H], DT, kind="ExternalOutput")

    a_f32r = a_dram[:].bitcast(F32R)

    with tile.TileContext(nc) as tc:
        with (
            tc.tile_pool(name="const", bufs=1) as const_pool,
            tc.tile_pool(name="hpool", bufs=1) as h_pool,
            tc.tile_pool(name="apanel", bufs=6) as a_pool,
            tc.tile_pool(name="atT", bufs=4) as at_pool,
            tc.tile_pool(name="tp", bufs=5, space=bass.MemorySpace.PSUM) as tp_pool,
            tc.tile_pool(name="acc", bufs=3, space=bass.MemorySpace.PSUM) as acc_pool,
            tc.tile_pool(name="outsb", bufs=4) as out_pool,
        ):
            # h first. The host hands us h partition-major ([128, 16*256]
            # contiguous, row p = rows {t*128+p} of the original), so the
            # two half-DMAs use fat 8KB-per-partition descriptor runs.
            h_flat = h_pool.tile([P, NT * H], DT)
            h_bf = h_pool.tile([P, NT, HE], BF16)
            nc.vector.memset(h_bf[:, :, H:HE], 1.0)
            nc.sync.dma_start(out=h_flat[:, 0 : NT * H // 2], in_=h_dram[:].rearrange("(p two) n -> p (two n)", p=P)[:, 0 : NT * H // 2])
            nc.scalar.dma_start(out=h_flat[:, NT * H // 2 :], in_=h_dram[:].rearrange("(p two) n -> p (two n)", p=P)[:, NT * H // 2 :])
            half = NT // 2
            h_flat_t = h_flat[:].rearrange("p (t n) -> p t n", n=H)
            nc.vector.tensor_copy(h_bf[:, 0:half, 0:H], h_flat_t[:, 0:half, :])
            nc.scalar.copy(h_bf[:, half:NT, 0:H], h_flat_t[:, half:NT, :])

            id_src = const_pool.tile([P, P], DT)
            make_identity(nc, id_src[:])
            identity = const_pool.tile([P, P], F32R)
            nc.vector.tensor_copy(identity[:], id_src[:])
            # Mask for the transposed diagonal block ([i(part), j(free)],
            # keep i >= j -> lower triangular).
            tmask = const_pool.tile([P, P], DT)
            make_lower_triangular(nc, tmask[:], val=1.0, diag=True)

            copy_eng = [
                lambda dst, src: nc.vector.tensor_copy(dst, src),
                lambda dst, src: nc.scalar.copy(dst, src),
            ]

            def store(jc, acc):
                out_sb = out_pool.tile([P, 2 * H], DT, tag="outsb")
                nc.scalar.copy(out_sb[:, 0:H], acc[:, 0:H])
                nc.vector.tensor_scalar(
                    out_sb[:, H : 2 * H],
                    h_flat[:, jc * H : (jc + 1) * H],
                    acc[:, H : H + 1],
                    None,
                    mybir.AluOpType.mult,
                )
                nc.gpsimd.dma_start(out_dram[jc * P : (jc + 1) * P, :], out_sb[:])

            ci = 0  # copy-engine round robin
            ring_i = 0  # A-chunk ring round robin
            a_rings = [nc.sync, nc.scalar]
            pending = []  # (jc, k0, tn, atT, acc, ntiles) awaiting matmuls
            done_mm = {}  # jc -> blocks matmul'd
            store_q = []  # completed panels awaiting store emission

            def flush_one():
                jc, k0, tn, atT, acc, ntiles = pending.pop(0)
                for k in range(tn):
                    nc.tensor.matmul(
                        acc[:],
                        atT[:, (k0 + k) * P : (k0 + k + 1) * P],
                        h_bf[:, jc + k0 + k, :],
                        start=(k0 + k == 0),
                        stop=(k0 + k == ntiles - 1),
                    )
                done_mm[jc] = done_mm.get(jc, 0) + tn
                if done_mm[jc] == ntiles:
                    store_q.append((jc, acc))

            # Interleave big and small panels: [0,15,1,14,...,7,8]
            jc_order = []
            for i in range(NT // 2):
                jc_order += [i, NT - 1 - i]

            for jc in jc_order:
                ntiles = NT - jc
                W = ntiles * P
                atT = at_pool.tile([P, W], BF16, tag="atT")
                acc = acc_pool.tile([P, HE], DT, tag="acc")

                chunks = []
                g0 = 0
                while g0 < ntiles:
                    gn = min(DMA_CHUNK, ntiles - g0)
                    a_chunk = a_pool.tile([P, DMA_CHUNK * P], F32R, tag="apanel")
                    a_rings[ring_i % 2].dma_start(
                        a_chunk[:, 0 : gn * P],
                        a_f32r[
                            jc * P : (jc + 1) * P,
                            (jc + g0) * P : (jc + g0 + gn) * P,
                        ],
                    )
                    ring_i += 1
                    chunks.append((g0, gn, a_chunk))
                    g0 += gn

                for c0, cn, a_chunk in chunks:
                    for t0 in range(0, cn, TGROUP):
                        tn = min(TGROUP, cn - t0)
                        k0 = c0 + t0  # first block index within the panel
                        tp = tp_pool.tile([P, TGROUP * P], F32R, tag="tp")
                        for k in range(tn):
                            nc.tensor.transpose(
                                tp[:, k * P : (k + 1) * P],
                                a_chunk[:, (t0 + k) * P : (t0 + k + 1) * P],
                                identity[:],
                            )
                        if k0 == 0:
                            # diagonal block: mask i >= j during the move
                            nc.vector.tensor_tensor(
                                atT[:, 0:P], tp[:, 0:P], tmask[:],
                                mybir.AluOpType.mult,
                            )
                            if tn > 1:
                                copy_eng[ci % 2](
                                    atT[:, P : tn * P], tp[:, P : tn * P]
                                )
                                ci += 1
                        else:
                            copy_eng[ci % 2](
                                atT[:, k0 * P : (k0 + tn) * P], tp[:, 0 : tn * P]
                            )
                            ci += 1
                        pending.append((jc, k0, tn, atT, acc, ntiles))
                        while len(pending) > MM_LAG:
                            flush_one()
                # emit previous panels' stores after this panel's copies are
                # queued, so store waits don't head-of-line block them
                while store_q:
                    store(*store_q.pop(0))
            while pending:
                flush_one()
            while store_q:
                store(*store_q.pop(0))

    nc.finalize()
    return nc


def kernel(span_adjacency, bound_hidden):
    global LAST_RESULTS
    a = np.ascontiguousarray(np.asarray(span_adjacency, dtype=np.float32))
    h = np.ascontiguousarray(np.asarray(bound_hidden, dtype=np.float32))
    assert a.shape == (B, L, L) and h.shape == (B, L, H), (a.shape, h.shape)

    key = "full"
    if key not in _NC_CACHE:
        _NC_CACHE[key] = _build_nc()
    nc = _NC_CACHE[key]

    # partition-major h: hp[p, t*H + n] = h[b, t*P + p, n] -> [L//NT? ...]
    # shape fed to the kernel: [P, NT*H] flattened back to [L, H] row-major
    # equivalent expected by h_dram ([L, H] with rows = (p, t) pairs).
    hp = [
        np.ascontiguousarray(
            h[b].reshape(NT, P, H).transpose(1, 0, 2).reshape(L, H)
        )
        for b in range(B)
    ]
    in_maps = [{"a": a[b], "h": hp[b]} for b in range(B)]
    res = run_bass_kernel_spmd(
        nc,
        in_maps,
        core_ids=list(range(B)),
        trace=bool(os.environ.get("KERNEL_TRACE")),
    )
    LAST_RESULTS = res
    out = np.stack([res.results[b]["out"] for b in range(B)], axis=0)
    return out


# revision 17
# speedup vs baseline: 1.1139x; 1.1139x over previous
"""Trainium2 Bass kernel for nn_BoundarySeg (segment_reduce).

out[b, j, 0:H]   = sum_{i>=j} A[b, j, i] * h[b, i, :]
out[b, j, H:2H]  = h[b, j, :] * sum_{i>=j} A[b, j, i]

Shapes: A [8, 2048, 2048] f32, h [8, 2048, 256] f32 -> out [8, 2048, 512] f32.
Sharding: data-parallel over batch; core c computes batch c.

Per-core algorithm (L=2048 in 16 tiles of 128, H=256):
  - h loads first as 4 quarters on 4 DMA rings into [128(p), 16(t), 258]
    fp32 with a ones column at [.., 256] (rowsum falls out of the matmul
    as an extra column); V/S cast quarters to bf16 for the matmul rhs.
  - Panels processed in interleaved order [0,15,1,14,...] so per-panel
    work (and the store stream) stays even over time. Panel DMAs go in
    <=8-block f32r chunks alternating sync/scalar HWDGE rings.
  - PE transposes each 128x128 block (f32r in/out, f32r identity as the
    moving operand); PSUM->SBUF move casts to bf16 (V/S alternating,
    4-block groups = 1 PSUM bank); the diagonal block gets the i>=j
    mask in that move. Matmuls (acc += At^T @ h_bf, bf16, N=258) lag
    the transposes by MM_LAG groups so the in-order PE queue never
    waits on the cross-engine copies.
  - Store per panel (deferred one panel to avoid head-of-line blocks):
    first half copy PSUM->SBUF (ACT), second half = h[j,:] * acc[:,256]
    via DVE tensor_scalar reading the rowsum straight from PSUM; out
    DMA on the gpsimd SWDGE ring.
"""

import os
import sys

import numpy as np

sys.path.insert(0, "/opt/trn_rl_repo")

import concourse.bass as bass  # noqa: E402
import concourse.bacc as bacc  # noqa: E402
import concourse.tile as tile  # noqa: E402
from concourse import mybir  # noqa: E402
from concourse.bass_utils import run_bass_kernel_spmd  # noqa: E402
from concourse.masks import make_identity, make_lower_triangular  # noqa: E402

B, L, H = 8, 2048, 256
P = 128
NT = L // P
HE = H + 2  # even N; col H = ones (rowsum), col H+1 unused
DMA_CHUNK = 8  # blocks per A-panel DMA
TGROUP = 4  # blocks per PE-transpose PSUM tile / copy (1 PSUM bank)
MM_LAG = 3  # groups the matmuls trail the transposes by

DT = mybir.dt.float32
F32R = mybir.dt.float32r
BF16 = mybir.dt.bfloat16

LAST_RESULTS = None
_NC_CACHE = {}


def _build_nc():
    nc = bacc.Bacc(None, target_bir_lowering=False)
    a_dram = nc.dram_tensor("a", [L, L], DT, kind="ExternalInput")
    h_dram = nc.dram_tensor("h", [L, H], DT, kind="ExternalInput")
    out_dram = nc.dram_tensor("out", [L, 2 * H], DT, kind="ExternalOutput")

    a_f32r = a_dram[:].bitcast(F32R)

    with tile.TileContext(nc) as tc:
        with (
            tc.tile_pool(name="const", bufs=1) as const_pool,
            tc.tile_pool(name="hpool", bufs=1) as h_pool,
            tc.tile_pool(name="apanel", bufs=6) as a_pool,
            tc.tile_pool(name="atT", bufs=4) as at_pool,
            tc.tile_pool(name="tp", bufs=5, space=bass.MemorySpace.PSUM) as tp_pool,
            tc.tile_pool(name="acc", bufs=3, space=bass.MemorySpace.PSUM) as acc_pool,
            tc.tile_pool(name="outsb", bufs=4) as out_pool,
        ):
            # h first. The host hands us h partition-major ([128, 16*256]
            # contiguous, row p = rows {t*128+p} of the original), so the
            # two half-DMAs use fat 8KB-per-partition descriptor runs.
            h_flat = h_pool.tile([P, NT * H], DT)
            h_bf = h_pool.tile([P, NT, HE], BF16)
            nc.vector.memset(h_bf[:, :, H:HE], 1.0)
            nc.gpsimd.dma_start(
                out=h_flat[:], in_=h_dram[:].rearrange("(p two) n -> p (two n)", p=P)
            )
            half = NT // 2
            h_flat_t = h_flat[:].rearrange("p (t n) -> p t n", n=H)
            nc.vector.tensor_copy(h_bf[:, 0:half, 0:H], h_flat_t[:, 0:half, :])
            nc.scalar.copy(h_bf[:, half:NT, 0:H], h_flat_t[:, half:NT, :])

            id_src = const_pool.tile([P, P], DT)
            make_identity(nc, id_src[:])
            identity = const_pool.tile([P, P], F32R)
            nc.vector.tensor_copy(identity[:], id_src[:])
            # Mask for the transposed diagonal block ([i(part), j(free)],
            # keep i >= j -> lower triangular).
            tmask = const_pool.tile([P, P], DT)
            make_lower_triangular(nc, tmask[:], val=1.0, diag=True)

            copy_eng = [
                lambda dst, src: nc.vector.tensor_copy(dst, src),
                lambda dst, src: nc.scalar.copy(dst, src),
            ]

            def store(jc, acc):
                out_sb = out_pool.tile([P, 2 * H], DT, tag="outsb")
                nc.scalar.copy(out_sb[:, 0:H], acc[:, 0:H])
                nc.vector.tensor_scalar(
                    out_sb[:, H : 2 * H],
                    h_flat[:, jc * H : (jc + 1) * H],
                    acc[:, H : H + 1],
                    None,
                    mybir.AluOpType.mult,
                )
                if jc % 2 == 0:
                    nc.gpsimd.dma_start(out_dram[jc * P : (jc + 1) * P, :], out_sb[:])
                else:
                    nc.scalar.dma_start(out_dram[jc * P : (jc + 1) * P, :], out_sb[:])

            ci = 0  # copy-engine round robin
            pending = []  # (jc, k0, tn, atT, acc, ntiles) awaiting matmuls
            done_mm = {}  # jc -> blocks matmul'd
            store_q = []  # completed panels awaiting store emission

            def flush_one():
                jc, k0, tn, atT, acc, ntiles = pending.pop(0)
                for k in range(tn):
                    nc.tensor.matmul(
                        acc[:],
                        atT[:, (k0 + k) * P : (k0 + k + 1) * P],
                        h_bf[:, jc + k0 + k, :],
                        start=(k0 + k == 0),
                        stop=(k0 + k == ntiles - 1),
                    )
                done_mm[jc] = done_mm.get(jc, 0) + tn
                if done_mm[jc] == ntiles:
                    store_q.append((jc, acc))

            # Interleave big and small panels: [0,15,1,14,...,7,8]
            jc_order = []
            for i in range(NT // 2):
                jc_order += [i, NT - 1 - i]

            for jc in jc_order:
                ntiles = NT - jc
                W = ntiles * P
                atT = at_pool.tile([P, W], BF16, tag="atT")
                acc = acc_pool.tile([P, HE], DT, tag="acc")

                chunks = []
                g0 = 0
                while g0 < ntiles:
                    gn = min(DMA_CHUNK, ntiles - g0)
                    a_chunk = a_pool.tile([P, DMA_CHUNK * P], F32R, tag="apanel")
                    nc.sync.dma_start(
                        a_chunk[:, 0 : gn * P],
                        a_f32r[
                            jc * P : (jc + 1) * P,
                            (jc + g0) * P : (jc + g0 + gn) * P,
                        ],
                    )
                    chunks.append((g0, gn, a_chunk))
                    g0 += gn

                for c0, cn, a_chunk in chunks:
                    for t0 in range(0, cn, TGROUP):
                        tn = min(TGROUP, cn - t0)
                        k0 = c0 + t0  # first block index within the panel
                        tp = tp_pool.tile([P, TGROUP * P], F32R, tag="tp")
                        for k in range(tn):
                            nc.tensor.transpose(
                                tp[:, k * P : (k + 1) * P],
                                a_chunk[:, (t0 + k) * P : (t0 + k + 1) * P],
                                identity[:],
                            )
                        if k0 == 0:
                            # diagonal block: mask i >= j during the move
                            nc.vector.tensor_tensor(
                                atT[:, 0:P], tp[:, 0:P], tmask[:],
                                mybir.AluOpType.mult,
                            )
                            if tn > 1:
                                copy_eng[ci % 2](
                                    atT[:, P : tn * P], tp[:, P : tn * P]
                                )
                                ci += 1
                        else:
                            copy_eng[ci % 2](
                                atT[:, k0 * P : (k0 + tn) * P], tp[:, 0 : tn * P]
                            )
                            ci += 1
                        pending.append((jc, k0, tn, atT, acc, ntiles))
                        while len(pending) > MM_LAG:
                            flush_one()
                # emit previous panels' stores after this panel's copies are
                # queued, so store waits don't head-of-line block them
                while store_q:
                    store(*store_q.pop(0))
            while pending:
                flush_one()
            while store_q:
                store(*store_q.pop(0))

    nc.finalize()
    return nc


def kernel(span_adjacency, bound_hidden):
    global LAST_RESULTS
    a = np.ascontiguousarray(np.asarray(span_adjacency, dtype=np.float32))
    h = np.ascontiguousarray(np.asarray(bound_hidden, dtype=np.float32))
    assert a.shape == (B, L, L) and h.shape == (B, L, H), (a.shape, h.shape)

    key = "full"
    if key not in _NC_CACHE:
        _NC_CACHE[key] = _build_nc()
    nc = _NC_CACHE[key]

    # partition-major h: hp[p, t*H + n] = h[b, t*P + p, n] -> [L//NT? ...]
    # shape fed to the kernel: [P, NT*H] flattened back to [L, H] row-major
    # equivalent expected by h_dram ([L, H] with rows = (p, t) pairs).
    hp = [
        np.ascontiguousarray(
            h[b].reshape(NT, P, H).transpose(1, 0, 2).reshape(L, H)
        )
        for b in range(B)
    ]
    in_maps = [{"a": a[b], "h": hp[b]} for b in range(B)]
    res = run_bass_kernel_spmd(
        nc,
        in_maps,
        core_ids=list(range(B)),
        trace=bool(os.environ.get("KERNEL_TRACE")),
    )
    LAST_RESULTS = res
    out = np.stack([res.results[b]["out"] for b in range(B)], axis=0)
    return out
